# revision 16
# baseline (speedup 1.0000x reference)
"""BitLinear (bit-decoded weights + STE quant) Trainium2 kernel.

y = x @ W^T + b, where
  W = decode_bits(bweight, wsign) * scale,  b = decode_bits(bbias, bsign)
      * biasscale;  decode: n = sum_k bits[..., k] * 2^(7-k) (exact ints
      0..255), W = n * (scale/255) * sign(wsign).

Device strategy per core (tensor-parallel 2 token-groups x 4 out-groups):
  - bits stream in as fp8 and are widened to fp16 by a casting SWDGE DMA,
    so every level of the decode tree runs in the DVE 2x perf mode.
  - decode is interleaved with the matmul: as each contraction block kb
    is decoded, 8 PSUM accumulation chains (2 token-groups x 4 out-blocks)
    immediately consume it, so the PE streams during decode instead of
    waiting for the full weight tensor.
  - eviction on the Scalar engine: y^T = Identity(psum * (scale/255)
    + bias_o) in fp16; host reassembles/upcasts.

All of the module's arithmetic (bit decode, sign, scaling, matmul, bias)
runs on the device; host-side work is layout/precision only.
"""

import numpy as np

import concourse.mybir as mybir
import concourse.tile as tile
from concourse import bacc
from concourse import bass_utils

# ---- problem constants (hardcoded per contract) ----
B, S, IN, OUT, NB = 4, 2048, 2048, 2048, 8
T = B * S                      # 8192 tokens
P = 128                        # partitions
P_T, P_O = 2, 4                # token-parallel x out-feature-parallel grid
N_CORES = P_T * P_O
T_SH = T // P_T                # 4096
O_SH = OUT // P_O              # 512
KB = IN // P                   # 16 contraction blocks
OB = O_SH // P                 # 4 out blocks
TGW = 512                      # t-group width
TG = T_SH // TGW               # 8 t-groups per core
XC = 4                         # x chunks per t-group (kb-granularity 4)
KPC = KB // XC                 # kb per x chunk

F32 = mybir.dt.float32
FP16 = mybir.dt.float16
BF16 = mybir.dt.bfloat16
FP8 = mybir.dt.float8e4
AL = mybir.AluOpType
IDENT = mybir.ActivationFunctionType.Identity

_CACHE = {}


def _pairs(ap):
    """Split the last (fast) axis of a [..., 2n] AP into even/odd views."""
    v = ap.rearrange("p (c two) -> p c two", two=2)
    return v[:, :, 0], v[:, :, 1]


def _build_nc(repeats=1):
    nc = bacc.Bacc("TRN2", target_bir_lowering=False, debug=False,
                   num_devices=N_CORES)

    xT = nc.dram_tensor("xT", [IN, T_SH], FP16, kind="ExternalInput").ap()
    bits = nc.dram_tensor("bits", [IN, O_SH * NB], FP8, kind="ExternalInput").ap()
    ws = nc.dram_tensor("ws", [IN, O_SH], BF16, kind="ExternalInput").ap()
    bb = nc.dram_tensor("bb", [O_SH, NB], F32, kind="ExternalInput").ap()
    bs = nc.dram_tensor("bs", [O_SH, 1], F32, kind="ExternalInput").ap()
    scl = nc.dram_tensor("scl", [P, 1], F32, kind="ExternalInput").ap()
    bscl = nc.dram_tensor("bscl", [P, 1], F32, kind="ExternalInput").ap()
    y = nc.dram_tensor("y", [O_SH, T_SH], FP16, kind="ExternalOutput").ap()

    bits3 = bits.rearrange("(kb p) c -> p kb c", p=P)
    ws3 = ws.rearrange("(kb p) o -> p kb o", p=P)
    xT4 = xT.rearrange("(kb p) t -> p kb t", p=P)
    y4 = y.rearrange("(ob p) t -> p ob t", p=P)

    with tile.TileContext(nc) as tc:
        with tc.tile_pool(name="const", bufs=1) as const, \
             tc.tile_pool(name="wt", bufs=2) as wt_pool, \
             tc.tile_pool(name="bt", bufs=4) as bt_pool, \
             tc.tile_pool(name="xs", bufs=12) as xs, \
             tc.tile_pool(name="dec", bufs=2) as dec, \
             tc.tile_pool(name="yb", bufs=2) as yb_pool, \
             tc.tile_pool(name="psum", bufs=1, space="PSUM") as psum_pool:

          for _rep in range(repeats):
            # ---- bits loads.  Host layout per kb row:
            #   [b0|b4|b2|b6 || b1|b5|b3|b7]  (blocks of O_SH, MSB=b0)
            # so every decode level is a contiguous step-1 fp16 op (DVE 2x).
            # Even-plane half arrives fp16 via SWDGE cast-DMA; odd-plane
            # half arrives raw fp8 on HWDGE and is widened on ACT. ----
            e16s, o8s = [], []

            def load_bits(kb):
                e16 = bt_pool.tile([P, O_SH * 4], FP16, tag="e16",
                                   name=f"e16_{kb}")
                nc.gpsimd.dma_start(out=e16, in_=bits3[:, kb, :O_SH * 4])
                e16s.append(e16)
                o8 = bt_pool.tile([P, O_SH * 4], FP8, tag="o8",
                                  name=f"o8_{kb}")
                nc.sync.dma_start(out=o8, in_=bits3[:, kb, O_SH * 4:])
                o8s.append(o8)

            for kb in range(3):
                load_bits(kb)

            # ---- ws (bf16, sign source), 4 separate chunk tiles ----
            ws_tiles = []
            for c in range(4):
                wst = const.tile([P, (KB // 4) * O_SH], BF16,
                                 name=f"ws_sb{c}")
                nc.sync.dma_start(
                    out=wst.rearrange("p (kb o) -> p kb o", kb=KB // 4),
                    in_=ws3[:, c * (KB // 4):(c + 1) * (KB // 4)])
                ws_tiles.append(wst)

            def ws_col(kb):
                wst = ws_tiles[kb // (KB // 4)]
                return wst.rearrange("p (kb o) -> p kb o",
                                     kb=KB // 4)[:, kb % (KB // 4)]

            # ---- x chunks: [P, KPC*TGW] fp16 tiles ----
            xts = {}

            def load_xg(g):
                for c in range(XC):
                    xt = xs.tile([P, KPC * TGW], FP16, tag="xg",
                                 name=f"xg{g}_{c}")
                    nc.sync.dma_start(
                        out=xt.rearrange("p (kb t) -> p kb t", kb=KPC),
                        in_=xT4[:, c * KPC:(c + 1) * KPC,
                                g * TGW:(g + 1) * TGW])
                    xts[(g, c)] = xt

            def xg_rhs(g, kb):
                xt = xts[(g, kb // KPC)]
                x3 = xt.rearrange("p (kb t) -> p kb t", kb=KPC)
                return x3[:, kb % KPC]

            load_xg(0)
            load_xg(1)

            # ---- scalars ----
            scl_sb = const.tile([P, 1], F32, name="scl_sb")
            nc.sync.dma_start(out=scl_sb, in_=scl)
            bscl_sb = const.tile([P, 1], F32, name="bscl_sb")
            nc.sync.dma_start(out=bscl_sb, in_=bscl)
            s255 = const.tile([P, 1], F32, name="s255")
            nc.vector.tensor_scalar_mul(s255, scl_sb, 1.0 / 255.0)
            bs255 = const.tile([P, 1], F32, name="bs255")
            nc.vector.tensor_scalar_mul(bs255, bscl_sb, 1.0 / 255.0)

            # ---- bias decode: bias_col [128, OB] (o on partitions) ----
            bias_col = const.tile([P, OB], F32, name="bias_col")
            with tc.tile_pool(name="btmp", bufs=1) as btmp:
                bb_sb = btmp.tile([P, OB * NB], F32)
                nc.sync.dma_start(
                    out=bb_sb.rearrange("p (ob k) -> p ob k", ob=OB),
                    in_=bb.rearrange("(ob p) k -> p ob k", p=P))
                bs_sb = btmp.tile([P, OB], F32)
                nc.sync.dma_start(
                    out=bs_sb.rearrange("p (ob k) -> p ob k", k=1),
                    in_=bs.rearrange("(ob p) k -> p ob k", p=P))
                e, o = _pairs(bb_sb)
                bl1 = btmp.tile([P, OB * 4], F32)
                nc.vector.scalar_tensor_tensor(out=bl1, in0=e, scalar=2.0,
                                               in1=o, op0=AL.mult, op1=AL.add)
                e, o = _pairs(bl1)
                bl2 = btmp.tile([P, OB * 2], F32)
                nc.vector.scalar_tensor_tensor(out=bl2, in0=e, scalar=4.0,
                                               in1=o, op0=AL.mult, op1=AL.add)
                e, o = _pairs(bl2)
                bl3 = btmp.tile([P, OB], F32)
                nc.vector.scalar_tensor_tensor(out=bl3, in0=e, scalar=16.0,
                                               in1=o, op0=AL.mult, op1=AL.add)
                bsg = btmp.tile([P, OB], F32)
                nc.scalar.sign(bsg, bs_sb)
                nc.vector.scalar_tensor_tensor(out=bias_col, in0=bl3,
                                               scalar=bs255, in1=bsg,
                                               op0=AL.mult, op1=AL.mult)

            # ---- fused decode + frontier matmul (groups 0,1) ----
            ASET = [(g, ob) for g in (0, 1) for ob in range(OB)]
            ps = [psum_pool.tile([P, TGW], F32, tag="mm", bufs=8,
                                 name=f"psA{i}")
                  for i in range(len(ASET))]
            wts = []
            for kb in range(KB):
                # decode tree: fully contiguous fp16 ops (DVE 2x mode).
                # layouts: l1 = [l1_0|l1_2|l1_1|l1_3], l2 = [l2_0|l2_1]
                o16 = dec.tile([P, O_SH * 4], FP16, tag="o16")
                nc.scalar.copy(o16, o8s[kb])
                l1 = dec.tile([P, O_SH * 4], FP16, tag="l1")
                nc.vector.scalar_tensor_tensor(
                    out=l1, in0=e16s[kb], scalar=2.0, in1=o16,
                    op0=AL.mult, op1=AL.add)
                l2 = dec.tile([P, O_SH * 2], FP16, tag="l2")
                nc.vector.scalar_tensor_tensor(
                    out=l2, in0=l1[:, :O_SH * 2], scalar=4.0,
                    in1=l1[:, O_SH * 2:], op0=AL.mult, op1=AL.add)
                l3 = dec.tile([P, O_SH], FP16, tag="l3")
                nc.vector.scalar_tensor_tensor(
                    out=l3, in0=l2[:, :O_SH], scalar=16.0,
                    in1=l2[:, O_SH:], op0=AL.mult, op1=AL.add)
                sg = dec.tile([P, O_SH], FP16, tag="sg")
                nc.scalar.sign(sg, ws_col(kb))
                wt = wt_pool.tile([P, O_SH], FP16, tag=f"wt{kb}",
                                  name=f"wt{kb}")
                nc.vector.tensor_tensor(out=wt, in0=l3, in1=sg, op=AL.mult)
                wts.append(wt)
                if kb + 3 < KB:
                    load_bits(kb + 3)

                # frontier: 8 chains consume this kb immediately
                for i, (g, ob) in enumerate(ASET):
                    nc.tensor.matmul(
                        ps[i],
                        wt[:, ob * P:(ob + 1) * P],
                        xg_rhs(g, kb),
                        start=(kb == 0),
                        stop=(kb == KB - 1),
                    )
                if kb == KB // 2:
                    load_xg(2)
                elif kb == KB - 4:
                    load_xg(3)

            # evict frontier chains; alternate DVE/ACT so the first
            # PSUM banks free sooner for the post-phase chains
            for g in (0, 1):
                ybuf = yb_pool.tile([P, OB * TGW], FP16, tag="ybuf")
                yb3 = ybuf.rearrange("p (ob t) -> p ob t", ob=OB)
                for ob in range(OB):
                    if ob % 2 == 0:
                        nc.vector.tensor_scalar(
                            out=yb3[:, ob], in0=ps[g * OB + ob],
                            scalar1=s255, scalar2=bias_col[:, ob:ob + 1],
                            op0=AL.mult, op1=AL.add)
                    else:
                        nc.scalar.activation(
                            out=yb3[:, ob], in_=ps[g * OB + ob], func=IDENT,
                            bias=bias_col[:, ob:ob + 1], scale=s255)
                nc.sync.dma_start(
                    out=y4[:, :, g * TGW:(g + 1) * TGW], in_=yb3)

            # ---- remaining groups: full chains, decode already done ----
            for g in range(2, TG):
                if g + 2 < TG:
                    load_xg(g + 2)
                ybuf = yb_pool.tile([P, OB * TGW], FP16, tag="ybuf")
                yb3 = ybuf.rearrange("p (ob t) -> p ob t", ob=OB)
                for ob in range(OB):
                    pst = psum_pool.tile([P, TGW], F32, tag="mm", bufs=8)
                    for kb in range(KB):
                        nc.tensor.matmul(
                            pst,
                            wts[kb][:, ob * P:(ob + 1) * P],
                            xg_rhs(g, kb),
                            start=(kb == 0),
                            stop=(kb == KB - 1),
                        )
                    nc.scalar.activation(
                        out=yb3[:, ob], in_=pst, func=IDENT,
                        bias=bias_col[:, ob:ob + 1], scale=s255)
                    if g == TG - 1:
                        # drain the final group per-ob to shorten the tail
                        nc.sync.dma_start(
                            out=y4[:, ob, g * TGW:(g + 1) * TGW],
                            in_=yb3[:, ob])
                if g != TG - 1:
                    nc.sync.dma_start(
                        out=y4[:, :, g * TGW:(g + 1) * TGW], in_=yb3)

    nc.compile()
    return nc


def _shard_inputs(x, bweight, wsign, scale, bbias, bsign, biasscale):
    fp8_np = mybir.dt.np(FP8)
    bf16_np = mybir.dt.np(BF16)

    x2 = np.asarray(x, dtype=np.float32).reshape(T, IN)
    xT_full = np.ascontiguousarray(x2.T.astype(np.float16))       # [IN, T]
    bT = np.asarray(bweight, dtype=np.float32).transpose(1, 0, 2)  # [IN, OUT, 8]
    wT = np.asarray(wsign, dtype=np.float32).T                    # [IN, OUT]
    bbias = np.asarray(bbias, dtype=np.float32)
    bsign = np.asarray(bsign, dtype=np.float32)

    scl_rep = np.full((P, 1), np.asarray(scale).reshape(-1)[0], dtype=np.float32)
    bscl_rep = np.full((P, 1), np.asarray(biasscale).reshape(-1)[0],
                       dtype=np.float32)

    in_maps = []
    for c in range(N_CORES):
        t_grp, o_grp = c // P_O, c % P_O
        tsl = slice(t_grp * T_SH, (t_grp + 1) * T_SH)
        osl = slice(o_grp * O_SH, (o_grp + 1) * O_SH)
        in_maps.append({
            "xT": np.ascontiguousarray(xT_full[:, tsl]),
            "bits": np.ascontiguousarray(
                bT[:, osl, :][:, :, [0, 4, 2, 6, 1, 5, 3, 7]]
                .transpose(0, 2, 1)).astype(fp8_np).reshape(IN, O_SH * NB),
            "ws": np.ascontiguousarray(wT[:, osl]).astype(bf16_np),
            "bb": np.ascontiguousarray(bbias[osl]),
            "bs": np.ascontiguousarray(bsign[osl]).reshape(O_SH, 1),
            "scl": scl_rep,
            "bscl": bscl_rep,
        })
    return in_maps


def kernel(x, bweight, wsign, scale, bbias, bsign, biasscale):
    if "nc" not in _CACHE:
        _CACHE["nc"] = _build_nc()
    nc = _CACHE["nc"]
    in_maps = _shard_inputs(x, bweight, wsign, scale, bbias, bsign, biasscale)
    res = bass_utils.run_bass_kernel_spmd(
        nc, in_maps, core_ids=list(range(N_CORES)))
    Y = np.empty((T, OUT), dtype=np.float32)
    for c in range(N_CORES):
        t_grp, o_grp = c // P_O, c % P_O
        Y[t_grp * T_SH:(t_grp + 1) * T_SH,
          o_grp * O_SH:(o_grp + 1) * O_SH] = res.results[c]["y"].T.astype(
              np.float32)
    return Y.reshape(B, S, OUT)


# revision 17
# speedup vs baseline: 1.1573x; 1.1573x over previous
"""BitLinear (bit-decoded weights + STE quant) Trainium2 kernel.

y = x @ W^T + b, where
  W = decode_bits(bweight, wsign) * scale,  b = decode_bits(bbias, bsign)
      * biasscale;  decode: n = sum_k bits[..., k] * 2^(7-k) (exact ints
      0..255), W = n * (scale/255) * sign(wsign).

Device strategy per core (tensor-parallel 2 token-groups x 4 out-groups):
  - bits stream in as fp8 and are widened to fp16 by a casting SWDGE DMA,
    so every level of the decode tree runs in the DVE 2x perf mode.
  - decode is interleaved with the matmul: as each contraction block kb
    is decoded, 8 PSUM accumulation chains (2 token-groups x 4 out-blocks)
    immediately consume it, so the PE streams during decode instead of
    waiting for the full weight tensor.
  - eviction on the Scalar engine: y^T = Identity(psum * (scale/255)
    + bias_o) in fp16; host reassembles/upcasts.

All of the module's arithmetic (bit decode, sign, scaling, matmul, bias)
runs on the device; host-side work is layout/precision only.
"""

import numpy as np

import concourse.mybir as mybir
import concourse.tile as tile
from concourse import bacc
from concourse import bass_utils

# ---- problem constants (hardcoded per contract) ----
B, S, IN, OUT, NB = 4, 2048, 2048, 2048, 8
T = B * S                      # 8192 tokens
P = 128                        # partitions
P_T, P_O = 2, 4                # token-parallel x out-feature-parallel grid
N_CORES = P_T * P_O
T_SH = T // P_T                # 4096
O_SH = OUT // P_O              # 512
KB = IN // P                   # 16 contraction blocks
OB = O_SH // P                 # 4 out blocks
TGW = 512                      # t-group width
TG = T_SH // TGW               # 8 t-groups per core
XC = 4                         # x chunks per t-group (kb-granularity 4)
KPC = KB // XC                 # kb per x chunk

F32 = mybir.dt.float32
FP16 = mybir.dt.float16
BF16 = mybir.dt.bfloat16
FP8 = mybir.dt.float8e4
AL = mybir.AluOpType
IDENT = mybir.ActivationFunctionType.Identity

_CACHE = {}


def _pairs(ap):
    """Split the last (fast) axis of a [..., 2n] AP into even/odd views."""
    v = ap.rearrange("p (c two) -> p c two", two=2)
    return v[:, :, 0], v[:, :, 1]


def _build_nc(repeats=1):
    nc = bacc.Bacc("TRN2", target_bir_lowering=False, debug=False,
                   num_devices=N_CORES)

    xT = nc.dram_tensor("xT", [IN, T_SH], FP16, kind="ExternalInput").ap()
    bits = nc.dram_tensor("bits", [IN, O_SH * NB], FP8, kind="ExternalInput").ap()
    ws = nc.dram_tensor("ws", [IN, O_SH], BF16, kind="ExternalInput").ap()
    bb = nc.dram_tensor("bb", [O_SH, NB], F32, kind="ExternalInput").ap()
    bs = nc.dram_tensor("bs", [O_SH, 1], F32, kind="ExternalInput").ap()
    scl = nc.dram_tensor("scl", [P, 1], F32, kind="ExternalInput").ap()
    bscl = nc.dram_tensor("bscl", [P, 1], F32, kind="ExternalInput").ap()
    y = nc.dram_tensor("y", [O_SH, T_SH], FP16, kind="ExternalOutput").ap()

    bits3 = bits.rearrange("(kb p) c -> p kb c", p=P)
    ws3 = ws.rearrange("(kb p) o -> p kb o", p=P)
    xT4 = xT.rearrange("(kb p) t -> p kb t", p=P)
    y4 = y.rearrange("(ob p) t -> p ob t", p=P)

    with tile.TileContext(nc) as tc:
        with tc.tile_pool(name="const", bufs=1) as const, \
             tc.tile_pool(name="wt", bufs=2) as wt_pool, \
             tc.tile_pool(name="bt", bufs=6) as bt_pool, \
             tc.tile_pool(name="xs", bufs=12) as xs, \
             tc.tile_pool(name="dec", bufs=2) as dec, \
             tc.tile_pool(name="yb", bufs=2) as yb_pool, \
             tc.tile_pool(name="psum", bufs=1, space="PSUM") as psum_pool:

          for _rep in range(repeats):
            # ---- bits loads.  Host layout per kb row:
            #   [b0|b4|b2|b6 || b1|b5|b3|b7]  (blocks of O_SH, MSB=b0)
            # so every decode level is a contiguous step-1 fp16 op (DVE 2x).
            # Even-plane half arrives fp16 via SWDGE cast-DMA; odd-plane
            # half arrives raw fp8 on HWDGE and is widened on ACT. ----
            e16s, o8s = [], []

            def load_bits(kb):
                e16 = bt_pool.tile([P, O_SH * 4], FP16, tag="e16",
                                   name=f"e16_{kb}")
                nc.gpsimd.dma_start(out=e16, in_=bits3[:, kb, :O_SH * 4])
                e16s.append(e16)
                o8 = bt_pool.tile([P, O_SH * 4], FP8, tag="o8",
                                  name=f"o8_{kb}")
                nc.sync.dma_start(out=o8, in_=bits3[:, kb, O_SH * 4:])
                o8s.append(o8)

            for kb in range(4):
                load_bits(kb)

            # ---- ws (bf16, sign source), 4 separate chunk tiles ----
            ws_tiles = []

            def load_ws(c):
                wst = const.tile([P, (KB // 4) * O_SH], BF16,
                                 name=f"ws_sb{c}")
                nc.sync.dma_start(
                    out=wst.rearrange("p (kb o) -> p kb o", kb=KB // 4),
                    in_=ws3[:, c * (KB // 4):(c + 1) * (KB // 4)])
                ws_tiles.append(wst)

            load_ws(0)

            def ws_col(kb):
                wst = ws_tiles[kb // (KB // 4)]
                return wst.rearrange("p (kb o) -> p kb o",
                                     kb=KB // 4)[:, kb % (KB // 4)]

            # ---- x chunks: [P, KPC*TGW] fp16 tiles ----
            xts = {}

            def load_xg(g):
                for c in range(XC):
                    xt = xs.tile([P, KPC * TGW], FP16, tag="xg",
                                 name=f"xg{g}_{c}")
                    nc.sync.dma_start(
                        out=xt.rearrange("p (kb t) -> p kb t", kb=KPC),
                        in_=xT4[:, c * KPC:(c + 1) * KPC,
                                g * TGW:(g + 1) * TGW])
                    xts[(g, c)] = xt

            def xg_rhs(g, kb):
                xt = xts[(g, kb // KPC)]
                x3 = xt.rearrange("p (kb t) -> p kb t", kb=KPC)
                return x3[:, kb % KPC]

            load_xg(0)
            load_bits(4)
            load_bits(5)
            load_xg(1)
            for c in range(1, 4):
                load_ws(c)

            # ---- scalars ----
            scl_sb = const.tile([P, 1], F32, name="scl_sb")
            nc.sync.dma_start(out=scl_sb, in_=scl)
            bscl_sb = const.tile([P, 1], F32, name="bscl_sb")
            nc.sync.dma_start(out=bscl_sb, in_=bscl)
            s255 = const.tile([P, 1], F32, name="s255")
            nc.vector.tensor_scalar_mul(s255, scl_sb, 1.0 / 255.0)
            bs255 = const.tile([P, 1], F32, name="bs255")
            nc.vector.tensor_scalar_mul(bs255, bscl_sb, 1.0 / 255.0)

            # ---- bias decode: bias_col [128, OB] (o on partitions) ----
            bias_col = const.tile([P, OB], F32, name="bias_col")
            with tc.tile_pool(name="btmp", bufs=1) as btmp:
                bb_sb = btmp.tile([P, OB * NB], F32)
                nc.sync.dma_start(
                    out=bb_sb.rearrange("p (ob k) -> p ob k", ob=OB),
                    in_=bb.rearrange("(ob p) k -> p ob k", p=P))
                bs_sb = btmp.tile([P, OB], F32)
                nc.sync.dma_start(
                    out=bs_sb.rearrange("p (ob k) -> p ob k", k=1),
                    in_=bs.rearrange("(ob p) k -> p ob k", p=P))
                e, o = _pairs(bb_sb)
                bl1 = btmp.tile([P, OB * 4], F32)
                nc.vector.scalar_tensor_tensor(out=bl1, in0=e, scalar=2.0,
                                               in1=o, op0=AL.mult, op1=AL.add)
                e, o = _pairs(bl1)
                bl2 = btmp.tile([P, OB * 2], F32)
                nc.vector.scalar_tensor_tensor(out=bl2, in0=e, scalar=4.0,
                                               in1=o, op0=AL.mult, op1=AL.add)
                e, o = _pairs(bl2)
                bl3 = btmp.tile([P, OB], F32)
                nc.vector.scalar_tensor_tensor(out=bl3, in0=e, scalar=16.0,
                                               in1=o, op0=AL.mult, op1=AL.add)
                bsg = btmp.tile([P, OB], F32)
                nc.scalar.sign(bsg, bs_sb)
                nc.vector.scalar_tensor_tensor(out=bias_col, in0=bl3,
                                               scalar=bs255, in1=bsg,
                                               op0=AL.mult, op1=AL.mult)

            # ---- fused decode + frontier matmul (groups 0,1) ----
            ASET = [(g, ob) for g in (0, 1) for ob in range(OB)]
            ps = [psum_pool.tile([P, TGW], F32, tag="mm", bufs=8,
                                 name=f"psA{i}")
                  for i in range(len(ASET))]
            wts = []
            for kb in range(KB):
                # decode tree: fully contiguous fp16 ops (DVE 2x mode).
                # layouts: l1 = [l1_0|l1_2|l1_1|l1_3], l2 = [l2_0|l2_1]
                o16 = dec.tile([P, O_SH * 4], FP16, tag="o16")
                nc.scalar.copy(o16, o8s[kb])
                l1 = dec.tile([P, O_SH * 4], FP16, tag="l1")
                nc.vector.scalar_tensor_tensor(
                    out=l1, in0=e16s[kb], scalar=2.0, in1=o16,
                    op0=AL.mult, op1=AL.add)
                l2 = dec.tile([P, O_SH * 2], FP16, tag="l2")
                nc.vector.scalar_tensor_tensor(
                    out=l2, in0=l1[:, :O_SH * 2], scalar=4.0,
                    in1=l1[:, O_SH * 2:], op0=AL.mult, op1=AL.add)
                l3 = dec.tile([P, O_SH], FP16, tag="l3")
                nc.vector.scalar_tensor_tensor(
                    out=l3, in0=l2[:, :O_SH], scalar=16.0,
                    in1=l2[:, O_SH:], op0=AL.mult, op1=AL.add)
                sg = dec.tile([P, O_SH], FP16, tag="sg")
                nc.scalar.sign(sg, ws_col(kb))
                wt = wt_pool.tile([P, O_SH], FP16, tag=f"wt{kb}",
                                  name=f"wt{kb}")
                nc.vector.tensor_tensor(out=wt, in0=l3, in1=sg, op=AL.mult)
                wts.append(wt)
                if kb + 6 < KB:
                    load_bits(kb + 6)

                # frontier: 8 chains consume this kb immediately
                for i, (g, ob) in enumerate(ASET):
                    nc.tensor.matmul(
                        ps[i],
                        wt[:, ob * P:(ob + 1) * P],
                        xg_rhs(g, kb),
                        start=(kb == 0),
                        stop=(kb == KB - 1),
                    )
                if kb == KB // 2:
                    load_xg(2)
                elif kb == KB - 4:
                    load_xg(3)

            # evict frontier chains; alternate DVE/ACT so the first
            # PSUM banks free sooner for the post-phase chains
            for g in (0, 1):
                ybuf = yb_pool.tile([P, OB * TGW], FP16, tag="ybuf")
                yb3 = ybuf.rearrange("p (ob t) -> p ob t", ob=OB)
                for ob in range(OB):
                    if ob % 2 == 0:
                        nc.vector.tensor_scalar(
                            out=yb3[:, ob], in0=ps[g * OB + ob],
                            scalar1=s255, scalar2=bias_col[:, ob:ob + 1],
                            op0=AL.mult, op1=AL.add)
                    else:
                        nc.scalar.activation(
                            out=yb3[:, ob], in_=ps[g * OB + ob], func=IDENT,
                            bias=bias_col[:, ob:ob + 1], scale=s255)
                nc.sync.dma_start(
                    out=y4[:, :, g * TGW:(g + 1) * TGW], in_=yb3)

            # ---- remaining groups: full chains, decode already done ----
            for g in range(2, TG):
                if g + 2 < TG:
                    load_xg(g + 2)
                ybuf = yb_pool.tile([P, OB * TGW], FP16, tag="ybuf")
                yb3 = ybuf.rearrange("p (ob t) -> p ob t", ob=OB)
                for ob in range(OB):
                    pst = psum_pool.tile([P, TGW], F32, tag="mm", bufs=8)
                    for kb in range(KB):
                        nc.tensor.matmul(
                            pst,
                            wts[kb][:, ob * P:(ob + 1) * P],
                            xg_rhs(g, kb),
                            start=(kb == 0),
                            stop=(kb == KB - 1),
                        )
                    nc.scalar.activation(
                        out=yb3[:, ob], in_=pst, func=IDENT,
                        bias=bias_col[:, ob:ob + 1], scale=s255)
                    if g == TG - 1:
                        # drain the final group per-ob to shorten the tail
                        nc.sync.dma_start(
                            out=y4[:, ob, g * TGW:(g + 1) * TGW],
                            in_=yb3[:, ob])
                if g != TG - 1:
                    nc.sync.dma_start(
                        out=y4[:, :, g * TGW:(g + 1) * TGW], in_=yb3)

    nc.compile()
    return nc


def _shard_inputs(x, bweight, wsign, scale, bbias, bsign, biasscale):
    fp8_np = mybir.dt.np(FP8)
    bf16_np = mybir.dt.np(BF16)

    x2 = np.asarray(x, dtype=np.float32).reshape(T, IN)
    xT_full = np.ascontiguousarray(x2.T.astype(np.float16))       # [IN, T]
    bT = np.asarray(bweight, dtype=np.float32).transpose(1, 0, 2)  # [IN, OUT, 8]
    wT = np.asarray(wsign, dtype=np.float32).T                    # [IN, OUT]
    bbias = np.asarray(bbias, dtype=np.float32)
    bsign = np.asarray(bsign, dtype=np.float32)

    scl_rep = np.full((P, 1), np.asarray(scale).reshape(-1)[0], dtype=np.float32)
    bscl_rep = np.full((P, 1), np.asarray(biasscale).reshape(-1)[0],
                       dtype=np.float32)

    in_maps = []
    for c in range(N_CORES):
        t_grp, o_grp = c // P_O, c % P_O
        tsl = slice(t_grp * T_SH, (t_grp + 1) * T_SH)
        osl = slice(o_grp * O_SH, (o_grp + 1) * O_SH)
        in_maps.append({
            "xT": np.ascontiguousarray(xT_full[:, tsl]),
            "bits": np.ascontiguousarray(
                bT[:, osl, :][:, :, [0, 4, 2, 6, 1, 5, 3, 7]]
                .transpose(0, 2, 1)).astype(fp8_np).reshape(IN, O_SH * NB),
            "ws": np.ascontiguousarray(wT[:, osl]).astype(bf16_np),
            "bb": np.ascontiguousarray(bbias[osl]),
            "bs": np.ascontiguousarray(bsign[osl]).reshape(O_SH, 1),
            "scl": scl_rep,
            "bscl": bscl_rep,
        })
    return in_maps


def kernel(x, bweight, wsign, scale, bbias, bsign, biasscale):
    if "nc" not in _CACHE:
        _CACHE["nc"] = _build_nc()
    nc = _CACHE["nc"]
    in_maps = _shard_inputs(x, bweight, wsign, scale, bbias, bsign, biasscale)
    res = bass_utils.run_bass_kernel_spmd(
        nc, in_maps, core_ids=list(range(N_CORES)))
    Y = np.empty((T, OUT), dtype=np.float32)
    for c in range(N_CORES):
        t_grp, o_grp = c // P_O, c % P_O
        Y[t_grp * T_SH:(t_grp + 1) * T_SH,
          o_grp * O_SH:(o_grp + 1) * O_SH] = res.results[c]["y"].T.astype(
              np.float32)
    return Y.reshape(B, S, OUT)
